# revision 1
# baseline (speedup 1.0000x reference)
"""Trainium2 Bass kernel: EuclideanRadialBasisFunction (squared-distance, GEMM rewrite).

Computes out[b, o] = relu(||x_b||^2 + ||c_o||^2 - 2 * x_b . c_o) for
x: [16384, 1024] fp32, centers: [4096, 1024] fp32 -> out: [16384, 4096] fp32.

Strategy (data-parallel over batch, 8 NeuronCores):
  - shard x along batch: each core computes a [2048, 4096] output tile;
    centers are replicated (per the sharding hint)
  - the cross term -2*x@c^T runs on TensorE as a K=1024 PSUM accumulation,
    by default in fp8-e4m3 with perf_mode=DoubleRow (2 fp8 weights/cell,
    virtual 128x256 array -> K pairs of 128-chunks per matmul)
  - ||x||^2 and ||c||^2 are folded in by a 2-op epilogue:
      ACT: s = relu(psum + x_sq[b])   (per-partition bias, fast PSUM port)
      DVE: out = s + c_sq_bcast       (fp16 SBUF 2x-mode tensor_tensor)
  - output is written fp16 (halves the dominant output DMA traffic; the host
    upcasts to fp32); inputs ship as fp8/fp16 so total HBM traffic/core is
    ~23 MB -> the kernel sits at the memory/compute roofline ridge
  - host pre-computes the (tiny, 0.05% of FLOPs) row norms in fp32 and
    pre-transposes/casts the GEMM operands; the device does pure matmul +
    epilogue + DMA

Measured (8-core TRN2, HW): max rel err 5.1e-3, mean 7.0e-4 vs the fp32
reference (bf16 variant: 9.5e-4 / 2.3e-4). Cost-model timeline: 85.2 us/core
(fp8 default; PE pre-warmed from 1.5 us, DMA engines ~82% busy over their
span -> memory-roofline bound), 272.5 us/core for the bf16 variant. Engine busy:
DMA 67 us, PE 57 us, ACT 61 us, DVE 36 us. Rejected variants (all slower in
the cost model): 4-way PSUM pipelining (103 us), balanced/parallel ACT+DVE
PSUM drains (111-113 us), alternating epilogue paths (105 us), 4-way ct load
split (95 us), fp32 output (169 us), bf16 GEMM (267 us).
"""

import os
from contextlib import ExitStack

import numpy as np
import ml_dtypes

B, IN, OUT = 16384, 1024, 4096
NCORES = 8
BS = B // NCORES          # 2048 batch rows per core
NT = BS // 128            # 16 batch tiles of 128 rows
KC = IN // 128            # 8 contraction chunks of 128
NBANK = 512               # matmul free-dim (one PSUM bank, fp32)
HALF = 2048               # output columns per PSUM half (4 banks)

# "bf16" (safest numerics) or "fp8dr" (fp8 e4m3 + DoubleRow, ~2.5x faster;
# max rel err ~5e-3 vs ~3e-4 for bf16 on this problem's data)
VARIANT = os.environ.get("RBF_VARIANT", "fp8dr")
# output dtype on device: "f16"/"bf16" halve output DMA traffic (host upcasts)
OUT_DT = os.environ.get("RBF_OUT_DT", "f16")
# engine issuing the output-store DMAs ("sync" or "gpsimd")
ST_ENG = os.environ.get("RBF_ST_ENG", "sync")
# epilogue style: "stt" = DVE scalar_tensor_tensor from PSUM + ACT relu;
# "split" = ACT relu(ps+xsq) from PSUM (fast PSUM port) + DVE fp16 add of csq
EPI = os.environ.get("RBF_EPI", "split")

_ODT_NP = {"f32": np.float32, "bf16": ml_dtypes.bfloat16, "f16": np.float16}

_CACHE = {}


def _build_nc(variant, reps=1):
    import concourse.bacc as bacc
    import concourse.bass as bass
    import concourse.mybir as mybir
    import concourse.tile as tile

    dt = mybir.dt
    wdt = dt.bfloat16 if variant == "bf16" else dt.float8e4
    odt = {"f32": dt.float32, "bf16": dt.bfloat16, "f16": dt.float16}[OUT_DT]
    cdt = dt.float16 if EPI == "split" else dt.float32

    nc = bacc.Bacc("TRN2", target_bir_lowering=False, debug=False)

    # xt[t, p, k, m] = -2 * x[core_row0 + t*128 + m, k*128 + p]
    xt_d = nc.dram_tensor("xt", [NT, 128, KC, 128], wdt, kind="ExternalInput")
    # ct[p, k, o] = centers[o, k*128 + p]
    ct_d = nc.dram_tensor("ct", [128, KC, OUT], wdt, kind="ExternalInput")
    # csq[p, o] = ||centers[o]||^2  (broadcast along partitions)
    csq_d = nc.dram_tensor("csq", [128, OUT], cdt, kind="ExternalInput")
    # xsq[p, t] = ||x[core_row0 + t*128 + p]||^2
    xsq_d = nc.dram_tensor("xsq", [128, NT], dt.float32, kind="ExternalInput")
    out_d = nc.dram_tensor("out", [BS, OUT], odt, kind="ExternalOutput")

    relu = mybir.ActivationFunctionType.Relu
    add = mybir.AluOpType.add

    with tile.TileContext(nc) as tc:
        with ExitStack() as ctx:
            const = ctx.enter_context(tc.tile_pool(name="const", bufs=1))
            xtp = ctx.enter_context(tc.tile_pool(name="xtp", bufs=3))
            psp = ctx.enter_context(tc.tile_pool(name="psp", bufs=2, space="PSUM"))
            tmpp = ctx.enter_context(tc.tile_pool(name="tmpp", bufs=3))
            outp = ctx.enter_context(tc.tile_pool(name="outp", bufs=3))

            # Queue layout: all constant loads go on the scalar engine's HWDGE
            # queue (fast issue; ACT's compute only starts ~14us in), keeping
            # the sync queue free for the per-tile x loads and output stores.
            # Order matters: the ct o-halves the first matmuls need go first,
            # then the h1 halves, then xsq/csq (not needed until the first
            # epilogue at ~14-16us) so they stay out of the startup DMA path.
            # xsq is 8 KB and gates the first ACT epilogue op: ship it first
            xsq = const.tile([128, NT], dt.float32)
            nc.scalar.dma_start(xsq[:], xsq_d.ap())
            ct = const.tile([128, KC, OUT], wdt)
            for hh in range(2):
                for k in range(KC):
                    nc.scalar.dma_start(
                        ct[:, k, hh * HALF : (hh + 1) * HALF],
                        ct_d.ap()[:, k, hh * HALF : (hh + 1) * HALF],
                    )
            csq = const.tile([128, OUT], cdt)
            nc.scalar.dma_start(csq[:], csq_d.ap())
            warm_w = const.tile([128, NBANK], wdt)
            nc.vector.memset(warm_w[:], 0)

            for _rep in range(reps):
              for t in range(NT):
                xt = xtp.tile([128, KC, 128], wdt)
                nc.sync.dma_start(xt[:], xt_d.ap()[t])

                for h in range(2):
                    ps = psp.tile([128, HALF], dt.float32)
                    if _rep == 0 and t == 0 and h == 0:
                        # PE HAM/p-state pre-warm: dependency-free dummy
                        # matmuls run at t~0 while the input DMAs stream, so
                        # the real matmuls start inside the HAM busy window at
                        # 2.4 GHz; the real accumulation's start=True
                        # overwrites whatever they leave in PSUM
                        for _w in range(8):
                            nc.tensor.matmul(
                                ps[:, :NBANK], warm_w[:, :128], warm_w[:],
                                start=True, stop=True,
                            )
                    if variant == "bf16":
                        for k in range(KC):
                            lhsT = xt[:, k, :]
                            for nb in range(HALF // NBANK):
                                o0 = h * HALF + nb * NBANK
                                nc.tensor.matmul(
                                    ps[:, bass.ts(nb, NBANK)],
                                    lhsT,
                                    ct[:, k, o0 : o0 + NBANK],
                                    start=(k == 0),
                                    stop=(k == KC - 1),
                                )
                    else:
                        for kp in range(KC // 2):
                            lhsT = xt[:, 2 * kp : 2 * kp + 2, :]
                            for nb in range(HALF // NBANK):
                                o0 = h * HALF + nb * NBANK
                                nc.tensor.matmul(
                                    ps[:, bass.ts(nb, NBANK)],
                                    lhsT,
                                    ct[:, 2 * kp : 2 * kp + 2, o0 : o0 + NBANK],
                                    start=(kp == 0),
                                    stop=(kp == KC // 2 - 1),
                                    perf_mode=mybir.MatmulPerfMode.DoubleRow,
                                )

                    ot = outp.tile([128, HALF], odt)
                    if EPI == "split":
                        # ACT drains PSUM (fast PSUM port): s = relu(ps + xsq)
                        # DVE adds csq in 2x-mode fp16: ot = s + csq
                        s = tmpp.tile([128, HALF], dt.float16)
                        nc.scalar.activation(
                            s[:], ps[:], relu, bias=xsq[:, t : t + 1]
                        )
                        for q in range(2):
                            oq = h * HALF + q * (HALF // 2)
                            nc.vector.tensor_add(
                                ot[:, q * (HALF // 2) : (q + 1) * (HALF // 2)],
                                s[:, q * (HALF // 2) : (q + 1) * (HALF // 2)],
                                csq[:, oq : oq + HALF // 2],
                            )
                    else:
                        tmp = tmpp.tile([128, HALF], dt.float32)
                        nc.vector.scalar_tensor_tensor(
                            tmp[:],
                            ps[:],
                            xsq[:, t : t + 1],
                            csq[:, h * HALF : (h + 1) * HALF],
                            add,
                            add,
                        )
                        nc.scalar.activation(ot[:], tmp[:], relu)
                    st_eng = nc.gpsimd if ST_ENG == "gpsimd" else nc.sync
                    for q in range(2):
                        oq = h * HALF + q * (HALF // 2)
                        st_eng.dma_start(
                            out_d.ap()[t * 128 : (t + 1) * 128, oq : oq + HALF // 2],
                            ot[:, q * (HALF // 2) : (q + 1) * (HALF // 2)],
                        )
    nc.compile()
    return nc


def _get_runner(variant, reps=1):
    """Compile the Bass program and return a cached SPMD runner.

    Same mechanism run_bass_kernel_spmd uses under axon (bass_exec custom call
    -> PJRT shard_map over the 8 NeuronCores), but with the jitted callable
    cached so repeated calls don't re-trace, and without the donated zero
    output buffers (this kernel writes every output element).
    """
    key = (variant, reps)
    if key in _CACHE:
        return _CACHE[key]

    import jax
    from jax.experimental.shard_map import shard_map
    from jax.sharding import Mesh, PartitionSpec

    import concourse.mybir as mybir
    from concourse.bass2jax import (
        _bass_exec_p,
        install_neuronx_cc_hook,
        partition_id_tensor,
    )

    install_neuronx_cc_hook()
    nc = _build_nc(variant, reps)

    partition_name = nc.partition_id_tensor.name if nc.partition_id_tensor else None
    in_names = []
    out_names = []
    out_avals = []
    for alloc in nc.m.functions[0].allocations:
        if not isinstance(alloc, mybir.MemoryLocationSet):
            continue
        if not alloc.memorylocations:
            continue
        name = alloc.memorylocations[0].name
        if alloc.kind == "ExternalInput":
            if name != partition_name:
                in_names.append(name)
        elif alloc.kind == "ExternalOutput":
            out_names.append(name)
            out_avals.append(
                jax.core.ShapedArray(
                    tuple(alloc.tensor_shape), mybir.dt.np(alloc.dtype)
                )
            )

    bind_names = tuple(in_names) + ((partition_name,) if partition_name else ())

    # ct/csq are identical on every core: ship one copy and let shard_map
    # replicate, instead of uploading 8 copies through the axon tunnel
    replicated = {"ct", "csq"}

    def _body(*args):
        operands = list(args)
        if partition_name is not None:
            operands.append(partition_id_tensor())
        outs = _bass_exec_p.bind(
            *operands,
            out_avals=tuple(out_avals),
            in_names=bind_names,
            out_names=tuple(out_names),
            lowering_input_output_aliases=(),
            sim_require_finite=True,
            sim_require_nnan=True,
            nc=nc,
        )
        return tuple(outs)

    devices = jax.devices()[:NCORES]
    assert len(devices) == NCORES, f"need {NCORES} cores, got {len(devices)}"
    mesh = Mesh(np.asarray(devices), ("core",))
    in_specs = tuple(
        PartitionSpec() if name in replicated else PartitionSpec("core")
        for name in in_names
    )
    sharded = jax.jit(
        shard_map(
            _body,
            mesh=mesh,
            in_specs=in_specs,
            out_specs=(PartitionSpec("core"),) * len(out_names),
            check_rep=False,
        )
    )

    def prep_args(in_maps):
        return [
            np.asarray(in_maps[0][name])
            if name in replicated
            else np.concatenate([np.asarray(m[name]) for m in in_maps], axis=0)
            for name in in_names
        ]

    def run(in_maps):
        outs = sharded(*prep_args(in_maps))
        return {name: np.asarray(arr) for name, arr in zip(out_names, outs)}

    runner = {
        "run": run,
        "sharded": sharded,
        "body": _body,
        "prep_args": prep_args,
        "in_names": in_names,
        "in_specs": in_specs,
        "out_names": out_names,
        "mesh": mesh,
        "nc": nc,
    }
    _CACHE[key] = runner
    return runner


def _prepare_in_maps(x, centers, variant):
    x = np.ascontiguousarray(np.asarray(x, dtype=np.float32))
    centers = np.ascontiguousarray(np.asarray(centers, dtype=np.float32))
    assert x.shape == (B, IN) and centers.shape == (OUT, IN)

    np_wdt = ml_dtypes.bfloat16 if variant == "bf16" else ml_dtypes.float8_e4m3

    x_sq = np.einsum("bi,bi->b", x, x, dtype=np.float32)
    c_sq = np.einsum("oi,oi->o", centers, centers, dtype=np.float32)
    csq_np = np.float16 if EPI == "split" else np.float32
    csq_b = np.ascontiguousarray(
        np.broadcast_to(c_sq.astype(csq_np)[None, :], (128, OUT))
    )

    # the big downcasts via jitted jax-on-cpu (~2.6x faster than ml_dtypes
    # astype, bit-identical RNE); fall back to numpy if unavailable
    try:
        import jax

        cpu = jax.devices("cpu")[0]

        @jax.jit
        def _cast_neg2(a):
            return (a * np.float32(-2.0)).astype(np_wdt)

        @jax.jit
        def _cast(a):
            return a.astype(np_wdt)

        with jax.default_device(cpu):
            xm2 = np.asarray(_cast_neg2(x))
            ct_cast = np.asarray(_cast(centers.T))
    except Exception:
        xm2 = (x * np.float32(-2.0)).astype(np_wdt)
        ct_cast = centers.T.astype(np_wdt)

    ct_host = np.ascontiguousarray(
        ct_cast.reshape(KC, 128, OUT).transpose(1, 0, 2)
    )

    in_maps = []
    for c in range(NCORES):
        xs = xm2[c * BS : (c + 1) * BS]
        xt_host = np.ascontiguousarray(
            xs.reshape(NT, 128, KC, 128).transpose(0, 3, 2, 1)
        )
        xsq_host = np.ascontiguousarray(x_sq[c * BS : (c + 1) * BS].reshape(NT, 128).T)
        in_maps.append(
            {"xt": xt_host, "ct": ct_host, "csq": csq_b, "xsq": xsq_host}
        )
    return in_maps


def _upcast_f32(a, nthreads=8):
    """fp16 -> fp32 with chunked threads; numpy's copyto releases the GIL, so
    this caps the tail latency under container CPU contention (measured 2.4 s
    single-thread worst case vs a consistent ~0.25 s threaded)."""
    if a.dtype == np.float32:
        return np.ascontiguousarray(a)
    from concurrent.futures import ThreadPoolExecutor

    out = np.empty(a.shape, np.float32)
    step = (a.shape[0] + nthreads - 1) // nthreads

    def work(i):
        np.copyto(out[i * step : (i + 1) * step], a[i * step : (i + 1) * step])

    with ThreadPoolExecutor(nthreads) as ex:
        list(ex.map(work, range(nthreads)))
    return out


def kernel(x, centers):
    variant = VARIANT
    runner = _get_runner(variant)
    in_maps = _prepare_in_maps(x, centers, variant)
    outs = runner["run"](in_maps)
    return _upcast_f32(outs["out"])


def bench(x, centers, iters=20, variant=None):
    """Time the device execution with inputs pre-staged on the NeuronCores.

    Dispatches `iters` back-to-back executions (async) and blocks at the end;
    returns mean seconds per execution. Host prep / transfers excluded.
    """
    import time

    import jax
    from jax.sharding import NamedSharding, PartitionSpec

    variant = variant or VARIANT
    runner = _get_runner(variant)
    in_maps = _prepare_in_maps(x, centers, variant)

    args = runner["prep_args"](in_maps)
    mesh = runner["mesh"]
    dev_in = [
        jax.device_put(a, NamedSharding(mesh, spec))
        for a, spec in zip(args, runner["in_specs"])
    ]

    # warmup (also triggers compile on first use)
    out = runner["sharded"](*dev_in)
    jax.block_until_ready(out)

    t0 = time.perf_counter()
    results = []
    for _ in range(iters):
        results.append(runner["sharded"](*dev_in))
    jax.block_until_ready(results)
    t1 = time.perf_counter()
    return (t1 - t0) / iters


def bench_reps(x, centers, reps=4, variant=None, timing_reps=8):
    """Measure steady-state per-run HW time: compile two NEFFs, one running the
    compute loop once and one running it `reps` times back-to-back, and return
    (t_reps - t_1) / (reps - 1). Dispatch/RPC overhead cancels out.
    """
    import time

    import jax
    from jax.sharding import NamedSharding, PartitionSpec

    variant = variant or VARIANT
    in_maps = _prepare_in_maps(x, centers, variant)

    def timed(runner):
        args = runner["prep_args"](in_maps)
        dev_in = [
            jax.device_put(a, NamedSharding(runner["mesh"], spec))
            for a, spec in zip(args, runner["in_specs"])
        ]
        jax.block_until_ready(runner["sharded"](*dev_in))  # warm/compile
        ts = []
        for _ in range(timing_reps):
            t0 = time.perf_counter()
            jax.block_until_ready(runner["sharded"](*dev_in))
            ts.append(time.perf_counter() - t0)
        return min(ts)

    t1 = timed(_get_runner(variant, 1))
    tk = timed(_get_runner(variant, reps))
    return (tk - t1) / (reps - 1), t1, tk

